# revision 25
# baseline (speedup 1.0000x reference)
"""BitLinear forward kernel for Trainium2 (8-core data-parallel SPMD).

Computes: out = activation_quant(simple_rms_norm(x)) @ (w_int8 * weight_scale).T + bias

Math notes (exactness):
  - q_int = round(x_norm * 127/absmax_norm) are integers in [-127, 127];
    w are integers in [-128, 127]. bf16 represents these exactly, products
    are <= 2^14 and row sums <= 2^24, so a bf16 matmul with fp32 PSUM
    accumulation is bit-exact integer arithmetic.
  - round-half-even is implemented with the magic-number trick:
    fp32 fma(x, c, 1.5*2^23) rounds x*c to the nearest integer (RNE),
    which matches jnp.round. The magic is subtracted afterwards.
  - the quantize multiplier is c = 127 * rms_inv / clip(absmax*rms_inv, eps)
    and the output scale is s_row = weight_scale * clip(absmax*rms_inv, eps)/127;
    out = (q_int @ w.T) * s_row + bias.
  - x is fed to the device as fp16 and the output is written as bf16;
    both are well inside the 2e-2 relative-error budget (measured ~4e-3)
    and halve the HBM traffic on each side.

Performance structure (vs the v1 baseline at 333 us):
  - The PE previously spent ~80 us on 512 explicit 128x128 transposes of
    the quantized activations (LDWEIGHTS transpose_mode + identity matmul
    pairs) plus PSUM->SBUF copies on ACT/DVE. Those are replaced by one
    DMA X-bar transpose per 128-row tile (SBUF->SBUF, blocked 3D output),
    leaving the PE with only the 1024 real 512-col matmuls (~220 us warm).
  - fp16 input + bf16 output keep total DMA (~70 MB incl. transposes)
    under the PE floor.

Sharding: x [8, 8192, 1024] is data-parallel over the batch dim, one batch
element (8192 rows) per NeuronCore; the 1024x1024 int8 weight, scale and
bias are replicated. No collectives needed.
"""

import sys
import types
from collections import deque
from contextlib import ExitStack

import numpy as np

import concourse.bass as bass
import concourse.mybir as mybir
import concourse.tile as tile
from concourse import bacc, bass_utils
from concourse.alu_op_type import AluOpType

N_CORES = 8
P = 128          # partitions
D = 1024         # model dim (both in and out)
G = 4            # 128-row tiles per supertile
KCH = D // P     # contraction chunks (8)
DEPTH = 3        # supertile software-pipeline depth
MAGIC = 12582912.0   # 1.5 * 2**23: fp32 round-to-nearest-integer magic
EPS_RMS = 1e-6
EPS_ACT = 1e-5

F32 = mybir.dt.float32
F16 = mybir.dt.float16
BF16 = mybir.dt.bfloat16


def install_ntff_hook():
    """Register the axon NTFF profiling hook (missing antenv.axon_hooks shim).

    Harmless if profiling is never requested; lets trace=True produce
    exec_time_ns under axon.
    """
    try:
        from antenv import axon_hooks  # noqa: F401
        return  # already present
    except ImportError:
        pass
    try:
        import antenv
        from trn_agent_boot.trn_boot import _ntff_profile_via_ctypes
    except ImportError:
        return
    mod = types.ModuleType("antenv.axon_hooks")
    holder = [None]
    mod.set_axon_ntff_profile_hook = lambda h: holder.__setitem__(0, h)
    mod.get_axon_ntff_profile_hook = lambda: holder[0]
    sys.modules["antenv.axon_hooks"] = mod
    antenv.axon_hooks = mod
    try:
        hook = _ntff_profile_via_ctypes("/opt/axon/libaxon_pjrt.so")
    except OSError:
        hook = None
    if hook is not None:
        mod.set_axon_ntff_profile_hook(hook)


def emit_bitlinear(ctx: ExitStack, tc: tile.TileContext, out: bass.AP, x: bass.AP,
                   wt: bass.AP, bias_d: bass.AP, ws127: bass.AP, rows: int):
    """Emit the per-core program. x/out are [rows, D] in DRAM (fp16 in /
    bf16 out); wt is the pre-transposed bf16 weight [D(d), D(o)]; ws127 is
    weight_scale/127 [1]."""
    nc = tc.nc
    n_super = rows // (G * P)
    X = mybir.AxisListType.X

    consts = ctx.enter_context(tc.tile_pool(name="consts", bufs=1))
    xpool = ctx.enter_context(tc.tile_pool(name="xin", bufs=DEPTH + 2))
    spool = ctx.enter_context(tc.tile_pool(name="stats", bufs=DEPTH + 1))
    scr = ctx.enter_context(tc.tile_pool(name="scratch", bufs=6))
    qpool = ctx.enter_context(tc.tile_pool(name="q", bufs=DEPTH))
    qtpool = ctx.enter_context(tc.tile_pool(name="qt", bufs=DEPTH + 2))
    opool = ctx.enter_context(tc.tile_pool(name="osb", bufs=3))
    po_pool = ctx.enter_context(tc.tile_pool(name="psum_o", bufs=4, space="PSUM"))

    xv = x.rearrange("(s g p) d -> s p g d", g=G, p=P)
    ov = out.rearrange("(s g p) d -> s p g d", g=G, p=P)

    x_prefetch = {}

    def issue_x(st):
        # Batched 1 MiB DMA per supertile on the GPSIMD (SWDGE) queue so
        # input loads never block the sync-queue transposes.  The first two
        # supertiles load per-g so the stats pipeline starts ~3 us earlier.
        if st >= n_super or st in x_prefetch:
            return
        xs = xpool.tile([P, G, D], F16, tag="xs")
        nc.gpsimd.dma_start(xs, xv[st])
        x_prefetch[st] = xs

    # x tiles for the first supertiles are issued before the weights so the
    # stats pipeline starts while the 2 MiB weight stream lands behind them.
    for _st0 in range(min(DEPTH, n_super)):
        issue_x(_st0)

    # Resident constants on the SWDGE queue, behind the first x tiles.
    wt_sb = consts.tile([P, KCH, D], BF16)
    nc.gpsimd.dma_start(wt_sb, wt.rearrange("(k p) o -> p k o", p=P))
    bias_sb = consts.tile([P, D], F32)
    nc.gpsimd.dma_start(bias_sb, bass.AP(tensor=bias_d.tensor, offset=bias_d.offset,
                                         ap=[[0, P]] + list(bias_d.ap)))
    ws_sb = consts.tile([P, 1], F32)
    nc.gpsimd.dma_start(ws_sb, ws127.to_broadcast([P, 1]))
    eps_sb = consts.tile([P, 1], F32)
    nc.vector.memset(eps_sb, EPS_RMS)
    magic_sb = consts.tile([P, 1], F32)
    nc.vector.memset(magic_sb, MAGIC)
    warm_sb = consts.tile([P, 1], F32)
    nc.scalar.activation(out=warm_sb, in_=magic_sb,
                         func=mybir.ActivationFunctionType.Sqrt)

    def stats_chain(absmax, ssq, cols):
        """Per-row scale math on [P, cols] stat tiles -> (srow, c4)."""
        # sqrt(mean(x^2) + eps) fused into one ACT op (affine + Sqrt), then
        # the production DVE reciprocal for rms_inv.
        sqv = spool.tile([P, cols], F32, tag="sqv")
        nc.scalar.activation(out=sqv, in_=ssq,
                             func=mybir.ActivationFunctionType.Sqrt,
                             bias=eps_sb[:, 0:1], scale=1.0 / D)
        rinv = spool.tile([P, cols], F32, tag="rinv")
        nc.vector.reciprocal(rinv, sqv)
        # vc = clip(absmax * rms_inv, eps_act)
        vn = spool.tile([P, cols], F32, tag="vn")
        nc.vector.tensor_mul(vn, absmax, rinv)
        vc = spool.tile([P, cols], F32, tag="vc")
        nc.vector.tensor_scalar_max(vc, vn, EPS_ACT)
        # s_row = vc * weight_scale/127
        srow = spool.tile([P, cols], F32, tag="srow")
        nc.vector.tensor_scalar_mul(srow, vc, ws_sb[:, 0:1])
        # c = 127 * rinv / vc
        rvc = spool.tile([P, cols], F32, tag="rvc")
        nc.vector.reciprocal(rvc, vc)
        c4a = spool.tile([P, cols], F32, tag="c4a")
        nc.vector.tensor_mul(c4a, rinv, rvc)
        c4 = spool.tile([P, cols], F32, tag="c4")
        nc.vector.tensor_scalar_mul(c4, c4a, 127.0)
        return srow, c4

    def front_end(st):
        """DMA in + stats + quantize + DMA-transpose; returns (qt, srows)."""
        issue_x(st + DEPTH)
        xs = x_prefetch.pop(st)
        absmax = spool.tile([P, G], F32, tag="absmax")
        ssq = spool.tile([P, G], F32, tag="ssq")
        for g in range(G):
            # --- per-row stats over the free (d) axis ---
            nc.vector.tensor_reduce(out=absmax[:, g:g + 1], in_=xs[:, g, :], axis=X,
                                    op=AluOpType.max, apply_absolute_value=True)
            sq_scr = scr.tile([P, D], BF16, tag="sq")
            nc.scalar.activation(out=sq_scr, in_=xs[:, g, :],
                                 func=mybir.ActivationFunctionType.Square,
                                 accum_out=ssq[:, g:g + 1])
        srow, c4 = stats_chain(absmax, ssq, G)
        qb = qpool.tile([P, G * D], BF16, tag="qb")
        for g in range(G):
            # quantize: yq = x*c + MAGIC  (fp32 fma -> integer+MAGIC, RNE)
            yq = scr.tile([P, D], F32, tag="yq")
            nc.scalar.activation(out=yq, in_=xs[:, g, :],
                                 func=mybir.ActivationFunctionType.Identity,
                                 bias=magic_sb[:, 0:1], scale=c4[:, g:g + 1])
            nc.vector.tensor_scalar_add(qb[:, g * D:(g + 1) * D], yq, -MAGIC)
        # blocked transpose via DMA X-bar (one 1 MiB transfer per supertile):
        # qt[p, g*KCH + k, m] = qb[m, g*D + 128k + p]
        qt = qtpool.tile([P, G * KCH, P], BF16, tag="qt")
        if st == 0:
            # per-g for the pipeline-fill supertile: the PE can start on g0's
            # slice while g1..g3 still quantize.
            for g in range(G):
                nc.sync.dma_start(qt[:, g * KCH:(g + 1) * KCH, :],
                                  qb[:, g * D:(g + 1) * D], transpose=True)
        else:
            nc.sync.dma_start(qt, qb, transpose=True)
        return qt, [srow[:, g:g + 1] for g in range(G)]

    def back_end(st, qt, srows):
        """Matmuls + epilogue + DMA out for one supertile."""
        og = opool.tile([P, G, D], BF16, tag="og")
        for g in range(G):
            po = po_pool.tile([P, D], F32)
            for k in range(KCH):
                for nh in range(2):
                    nc.tensor.matmul(po[:, nh * 512:(nh + 1) * 512],
                                     qt[:, g * KCH + k, :],
                                     wt_sb[:, k, nh * 512:(nh + 1) * 512],
                                     start=(k == 0), stop=(k == KCH - 1))
            # epilogue: out = po * s_row + bias  (fused scalar_tensor_tensor)
            nc.vector.scalar_tensor_tensor(
                out=og[:, g, :], in0=po, scalar=srows[g], in1=bias_sb,
                op0=AluOpType.mult, op1=AluOpType.add)
        nc.gpsimd.dma_start(ov[st], og)

    # Software pipeline: run the front-end DEPTH supertiles ahead of the
    # back-end so the PE always has transposed activations ready.
    pending = deque()
    for st in range(n_super):
        pending.append(front_end(st))
        if len(pending) > DEPTH:
            back_end(st - DEPTH, *pending.popleft())
    for i, fe in enumerate(pending):
        back_end(n_super - len(pending) + i, *fe)


def build_program(rows: int = 8192):
    nc = bacc.Bacc("TRN2", target_bir_lowering=False, debug=False)
    x = nc.dram_tensor("x", [rows, D], F16, kind="ExternalInput").ap()
    wt = nc.dram_tensor("wt", [D, D], BF16, kind="ExternalInput").ap()
    bias_d = nc.dram_tensor("bias", [D], F32, kind="ExternalInput").ap()
    ws127 = nc.dram_tensor("ws127", [1], F32, kind="ExternalInput").ap()
    out = nc.dram_tensor("out", [rows, D], BF16, kind="ExternalOutput").ap()
    with tile.TileContext(nc) as tc:
        with ExitStack() as ctx:
            emit_bitlinear(ctx, tc, out, x, wt, bias_d, ws127, rows)
    nc.compile()
    return nc


_PROGRAM_CACHE = {}


def _get_program(rows: int):
    if rows not in _PROGRAM_CACHE:
        _PROGRAM_CACHE[rows] = build_program(rows)
    return _PROGRAM_CACHE[rows]


def prep_host_inputs(x, w_int8, weight_scale, bias):
    """Host-side prep: shard x over batch, pre-transpose/cast weights."""
    import ml_dtypes
    x = np.asarray(x, dtype=np.float32)
    w = np.asarray(w_int8)
    b, s, d = x.shape
    assert d == D and b == N_CORES
    x_f16 = x.astype(np.float16)
    wt_bf16 = np.ascontiguousarray(w.T).astype(ml_dtypes.bfloat16)  # [d, o], ints exact
    bias_f32 = np.asarray(bias, dtype=np.float32)
    ws127 = np.asarray([np.float32(weight_scale) / 127.0], dtype=np.float32)
    in_maps = []
    for c in range(N_CORES):
        in_maps.append({
            "x": np.ascontiguousarray(x_f16[c].reshape(s, d)),
            "wt": wt_bf16,
            "bias": bias_f32,
            "ws127": ws127,
        })
    return in_maps


def run(x, w_int8, weight_scale, bias, trace=False):
    """Run the SPMD kernel; returns (out [B,S,D] f32, BassKernelResults)."""
    b, s, d = np.asarray(x).shape
    nc = _get_program(s)
    in_maps = prep_host_inputs(x, w_int8, weight_scale, bias)
    if trace:
        install_ntff_hook()
    res = bass_utils.run_bass_kernel_spmd(
        nc, in_maps, core_ids=list(range(N_CORES)), trace=trace)
    out = np.stack([np.asarray(res.results[c]["out"]) for c in range(N_CORES)],
                   axis=0).astype(np.float32)
    return out.reshape(b, s, d), res


def kernel(x, w_int8, weight_scale, bias):
    out, _ = run(x, w_int8, weight_scale, bias, trace=False)
    return out


if __name__ == "__main__":
    # quick self-run with random data
    rng = np.random.default_rng(0)
    x = rng.standard_normal((N_CORES, 1024, D), dtype=np.float32)
    w = rng.integers(-128, 128, size=(D, D)).astype(np.int32)
    ws = np.float32(127.0 / 0.06)
    bias = (rng.standard_normal(D) * 0.01).astype(np.float32)
    out, res = run(x, w, ws, bias)
    print("out shape:", out.shape, "exec_time_ns:", res.exec_time_ns)


# revision 26
# speedup vs baseline: 1.0031x; 1.0031x over previous
"""BitLinear forward kernel for Trainium2 (8-core data-parallel SPMD).

Computes: out = activation_quant(simple_rms_norm(x)) @ (w_int8 * weight_scale).T + bias

Math notes (exactness):
  - q_int = round(x_norm * 127/absmax_norm) are integers in [-127, 127];
    w are integers in [-128, 127]. bf16 represents these exactly, products
    are <= 2^14 and row sums <= 2^24, so a bf16 matmul with fp32 PSUM
    accumulation is bit-exact integer arithmetic.
  - round-half-even is implemented with the magic-number trick:
    fp32 fma(x, c, 1.5*2^23) rounds x*c to the nearest integer (RNE),
    which matches jnp.round. The magic is subtracted afterwards.
  - the quantize multiplier is c = 127 * rms_inv / clip(absmax*rms_inv, eps)
    and the output scale is s_row = weight_scale * clip(absmax*rms_inv, eps)/127;
    out = (q_int @ w.T) * s_row + bias.
  - x is fed to the device as fp16 and the output is written as bf16;
    both are well inside the 2e-2 relative-error budget (measured ~4e-3)
    and halve the HBM traffic on each side.

Performance structure (vs the v1 baseline at 333 us):
  - The PE previously spent ~80 us on 512 explicit 128x128 transposes of
    the quantized activations (LDWEIGHTS transpose_mode + identity matmul
    pairs) plus PSUM->SBUF copies on ACT/DVE. Those are replaced by one
    DMA X-bar transpose per 128-row tile (SBUF->SBUF, blocked 3D output),
    leaving the PE with only the 1024 real 512-col matmuls (~220 us warm).
  - fp16 input + bf16 output keep total DMA (~70 MB incl. transposes)
    under the PE floor.

Sharding: x [8, 8192, 1024] is data-parallel over the batch dim, one batch
element (8192 rows) per NeuronCore; the 1024x1024 int8 weight, scale and
bias are replicated. No collectives needed.
"""

import sys
import types
from collections import deque
from contextlib import ExitStack

import numpy as np

import concourse.bass as bass
import concourse.mybir as mybir
import concourse.tile as tile
from concourse import bacc, bass_utils
from concourse.alu_op_type import AluOpType

N_CORES = 8
P = 128          # partitions
D = 1024         # model dim (both in and out)
G = 4            # 128-row tiles per supertile
KCH = D // P     # contraction chunks (8)
DEPTH = 3        # supertile software-pipeline depth
MAGIC = 12582912.0   # 1.5 * 2**23: fp32 round-to-nearest-integer magic
EPS_RMS = 1e-6
EPS_ACT = 1e-5

F32 = mybir.dt.float32
F16 = mybir.dt.float16
BF16 = mybir.dt.bfloat16


def install_ntff_hook():
    """Register the axon NTFF profiling hook (missing antenv.axon_hooks shim).

    Harmless if profiling is never requested; lets trace=True produce
    exec_time_ns under axon.
    """
    try:
        from antenv import axon_hooks  # noqa: F401
        return  # already present
    except ImportError:
        pass
    try:
        import antenv
        from trn_agent_boot.trn_boot import _ntff_profile_via_ctypes
    except ImportError:
        return
    mod = types.ModuleType("antenv.axon_hooks")
    holder = [None]
    mod.set_axon_ntff_profile_hook = lambda h: holder.__setitem__(0, h)
    mod.get_axon_ntff_profile_hook = lambda: holder[0]
    sys.modules["antenv.axon_hooks"] = mod
    antenv.axon_hooks = mod
    try:
        hook = _ntff_profile_via_ctypes("/opt/axon/libaxon_pjrt.so")
    except OSError:
        hook = None
    if hook is not None:
        mod.set_axon_ntff_profile_hook(hook)


def emit_bitlinear(ctx: ExitStack, tc: tile.TileContext, out: bass.AP, x: bass.AP,
                   wt: bass.AP, bias_d: bass.AP, ws127: bass.AP, rows: int):
    """Emit the per-core program. x/out are [rows, D] in DRAM (fp16 in /
    bf16 out); wt is the pre-transposed bf16 weight [D(d), D(o)]; ws127 is
    weight_scale/127 [1]."""
    nc = tc.nc
    n_super = rows // (G * P)
    X = mybir.AxisListType.X

    consts = ctx.enter_context(tc.tile_pool(name="consts", bufs=1))
    xpool = ctx.enter_context(tc.tile_pool(name="xin", bufs=DEPTH + 2))
    spool = ctx.enter_context(tc.tile_pool(name="stats", bufs=DEPTH + 1))
    scr = ctx.enter_context(tc.tile_pool(name="scratch", bufs=6))
    qpool = ctx.enter_context(tc.tile_pool(name="q", bufs=DEPTH))
    qtpool = ctx.enter_context(tc.tile_pool(name="qt", bufs=DEPTH + 2))
    opool = ctx.enter_context(tc.tile_pool(name="osb", bufs=3))
    po_pool = ctx.enter_context(tc.tile_pool(name="psum_o", bufs=4, space="PSUM"))

    xv = x.rearrange("(s g p) d -> s p g d", g=G, p=P)
    ov = out.rearrange("(s g p) d -> s p g d", g=G, p=P)

    x_prefetch = {}

    def issue_x(st):
        # Batched 1 MiB DMA per supertile on the GPSIMD (SWDGE) queue so
        # input loads never block the sync-queue transposes.  The first two
        # supertiles load per-g so the stats pipeline starts ~3 us earlier.
        if st >= n_super or st in x_prefetch:
            return
        xs = xpool.tile([P, G, D], F16, tag="xs")
        nc.gpsimd.dma_start(xs, xv[st])
        x_prefetch[st] = xs

    # x tiles for the first supertiles are issued before the weights so the
    # stats pipeline starts while the 2 MiB weight stream lands behind them.
    for _st0 in range(min(DEPTH, n_super)):
        issue_x(_st0)

    # Resident constants on the SWDGE queue, behind the first x tiles.
    wt_sb = consts.tile([P, KCH, D], BF16)
    nc.gpsimd.dma_start(wt_sb, wt.rearrange("(k p) o -> p k o", p=P))
    bias_sb = consts.tile([P, D], F32)
    nc.gpsimd.dma_start(bias_sb, bass.AP(tensor=bias_d.tensor, offset=bias_d.offset,
                                         ap=[[0, P]] + list(bias_d.ap)))
    ws_sb = consts.tile([P, 1], F32)
    nc.gpsimd.dma_start(ws_sb, ws127.to_broadcast([P, 1]))
    eps_sb = consts.tile([P, 1], F32)
    nc.vector.memset(eps_sb, EPS_RMS)
    magic_sb = consts.tile([P, 1], F32)
    nc.vector.memset(magic_sb, MAGIC)
    warm_sb = consts.tile([P, 1], F32)
    nc.scalar.activation(out=warm_sb, in_=magic_sb,
                         func=mybir.ActivationFunctionType.Sqrt)

    def stats_chain(absmax, ssq, cols):
        """Per-row scale math on [P, cols] stat tiles -> (srow, c4)."""
        # sqrt(mean(x^2) + eps) fused into one ACT op (affine + Sqrt), then
        # the production DVE reciprocal for rms_inv.
        sqv = spool.tile([P, cols], F32, tag="sqv")
        nc.scalar.activation(out=sqv, in_=ssq,
                             func=mybir.ActivationFunctionType.Sqrt,
                             bias=eps_sb[:, 0:1], scale=1.0 / D)
        rinv = spool.tile([P, cols], F32, tag="rinv")
        nc.vector.reciprocal(rinv, sqv)
        # vc = clip(absmax * rms_inv, eps_act)
        vn = spool.tile([P, cols], F32, tag="vn")
        nc.vector.tensor_mul(vn, absmax, rinv)
        vc = spool.tile([P, cols], F32, tag="vc")
        nc.vector.tensor_scalar_max(vc, vn, EPS_ACT)
        # s_row = vc * weight_scale/127
        srow = spool.tile([P, cols], F32, tag="srow")
        nc.vector.tensor_scalar_mul(srow, vc, ws_sb[:, 0:1])
        # c = 127 * rinv / vc
        rvc = spool.tile([P, cols], F32, tag="rvc")
        nc.vector.reciprocal(rvc, vc)
        c4a = spool.tile([P, cols], F32, tag="c4a")
        nc.vector.tensor_mul(c4a, rinv, rvc)
        c4 = spool.tile([P, cols], F32, tag="c4")
        nc.vector.tensor_scalar_mul(c4, c4a, 127.0)
        return srow, c4

    def front_end(st):
        """DMA in + stats + quantize + DMA-transpose; returns (qt, srows)."""
        issue_x(st + DEPTH)
        xs = x_prefetch.pop(st)
        absmax = spool.tile([P, G], F32, tag="absmax")
        ssq = spool.tile([P, G], F32, tag="ssq")
        for g in range(G):
            # --- per-row stats over the free (d) axis ---
            nc.vector.tensor_reduce(out=absmax[:, g:g + 1], in_=xs[:, g, :], axis=X,
                                    op=AluOpType.max, apply_absolute_value=True)
            sq_scr = scr.tile([P, D], BF16, tag="sq")
            nc.scalar.activation(out=sq_scr, in_=xs[:, g, :],
                                 func=mybir.ActivationFunctionType.Square,
                                 accum_out=ssq[:, g:g + 1])
        srow, c4 = stats_chain(absmax, ssq, G)
        qb = qpool.tile([P, G * D], BF16, tag="qb")
        for g in range(G):
            # quantize: yq = x*c + MAGIC  (fp32 fma -> integer+MAGIC, RNE)
            yq = scr.tile([P, D], F32, tag="yq")
            nc.scalar.activation(out=yq, in_=xs[:, g, :],
                                 func=mybir.ActivationFunctionType.Identity,
                                 bias=magic_sb[:, 0:1], scale=c4[:, g:g + 1])
            nc.vector.tensor_scalar_add(qb[:, g * D:(g + 1) * D], yq, -MAGIC)
        # blocked transpose via DMA X-bar (one 1 MiB transfer per supertile):
        # qt[p, g*KCH + k, m] = qb[m, g*D + 128k + p]
        qt = qtpool.tile([P, G * KCH, P], BF16, tag="qt")
        nc.sync.dma_start(qt, qb, transpose=True)
        return qt, [srow[:, g:g + 1] for g in range(G)]

    def back_end(st, qt, srows):
        """Matmuls + epilogue + DMA out for one supertile."""
        og = opool.tile([P, G, D], BF16, tag="og")
        for g in range(G):
            po = po_pool.tile([P, D], F32)
            for k in range(KCH):
                for nh in range(2):
                    nc.tensor.matmul(po[:, nh * 512:(nh + 1) * 512],
                                     qt[:, g * KCH + k, :],
                                     wt_sb[:, k, nh * 512:(nh + 1) * 512],
                                     start=(k == 0), stop=(k == KCH - 1))
            # epilogue: out = po * s_row + bias  (fused scalar_tensor_tensor)
            nc.vector.scalar_tensor_tensor(
                out=og[:, g, :], in0=po, scalar=srows[g], in1=bias_sb,
                op0=AluOpType.mult, op1=AluOpType.add)
        nc.gpsimd.dma_start(ov[st], og)

    # Software pipeline: run the front-end DEPTH supertiles ahead of the
    # back-end so the PE always has transposed activations ready.
    pending = deque()
    for st in range(n_super):
        pending.append(front_end(st))
        if len(pending) > DEPTH:
            back_end(st - DEPTH, *pending.popleft())
    for i, fe in enumerate(pending):
        back_end(n_super - len(pending) + i, *fe)


def build_program(rows: int = 8192):
    nc = bacc.Bacc("TRN2", target_bir_lowering=False, debug=False)
    x = nc.dram_tensor("x", [rows, D], F16, kind="ExternalInput").ap()
    wt = nc.dram_tensor("wt", [D, D], BF16, kind="ExternalInput").ap()
    bias_d = nc.dram_tensor("bias", [D], F32, kind="ExternalInput").ap()
    ws127 = nc.dram_tensor("ws127", [1], F32, kind="ExternalInput").ap()
    out = nc.dram_tensor("out", [rows, D], BF16, kind="ExternalOutput").ap()
    with tile.TileContext(nc) as tc:
        with ExitStack() as ctx:
            emit_bitlinear(ctx, tc, out, x, wt, bias_d, ws127, rows)
    nc.compile()
    return nc


_PROGRAM_CACHE = {}


def _get_program(rows: int):
    if rows not in _PROGRAM_CACHE:
        _PROGRAM_CACHE[rows] = build_program(rows)
    return _PROGRAM_CACHE[rows]


def prep_host_inputs(x, w_int8, weight_scale, bias):
    """Host-side prep: shard x over batch, pre-transpose/cast weights."""
    import ml_dtypes
    x = np.asarray(x, dtype=np.float32)
    w = np.asarray(w_int8)
    b, s, d = x.shape
    assert d == D and b == N_CORES
    x_f16 = x.astype(np.float16)
    wt_bf16 = np.ascontiguousarray(w.T).astype(ml_dtypes.bfloat16)  # [d, o], ints exact
    bias_f32 = np.asarray(bias, dtype=np.float32)
    ws127 = np.asarray([np.float32(weight_scale) / 127.0], dtype=np.float32)
    in_maps = []
    for c in range(N_CORES):
        in_maps.append({
            "x": np.ascontiguousarray(x_f16[c].reshape(s, d)),
            "wt": wt_bf16,
            "bias": bias_f32,
            "ws127": ws127,
        })
    return in_maps


def run(x, w_int8, weight_scale, bias, trace=False):
    """Run the SPMD kernel; returns (out [B,S,D] f32, BassKernelResults)."""
    b, s, d = np.asarray(x).shape
    nc = _get_program(s)
    in_maps = prep_host_inputs(x, w_int8, weight_scale, bias)
    if trace:
        install_ntff_hook()
    res = bass_utils.run_bass_kernel_spmd(
        nc, in_maps, core_ids=list(range(N_CORES)), trace=trace)
    out = np.stack([np.asarray(res.results[c]["out"]) for c in range(N_CORES)],
                   axis=0).astype(np.float32)
    return out.reshape(b, s, d), res


def kernel(x, w_int8, weight_scale, bias):
    out, _ = run(x, w_int8, weight_scale, bias, trace=False)
    return out


if __name__ == "__main__":
    # quick self-run with random data
    rng = np.random.default_rng(0)
    x = rng.standard_normal((N_CORES, 1024, D), dtype=np.float32)
    w = rng.integers(-128, 128, size=(D, D)).astype(np.int32)
    ws = np.float32(127.0 / 0.06)
    bias = (rng.standard_normal(D) * 0.01).astype(np.float32)
    out, res = run(x, w, ws, bias)
    print("out shape:", out.shape, "exec_time_ns:", res.exec_time_ns)
